# revision 38
# baseline (speedup 1.0000x reference)
"""Chamfer loss (nn_ChamferLoss) Trainium2 Bass kernel, v2.

Problem: x, y: [B=4, D=3, N=M=8192] fp32. Output: scalar
    dist = mean_b mean_n min_m d2[b,n,m] + mean_b mean_m min_n d2[b,n,m]
    d2 = |x_n|^2 + |y_m|^2 - 2 x_n.y_m

Strategy
--------
* Host: pre-round points to the PE's f32r format and augment to 7 dims so a
  single K=7 f32r matmul (1 cyc/row) emits exact squared distances between
  the rounded points:
    xa = [-2*xr, |xr|^2_hi, |xr|^2_lo, 1, 1]
    ya = [ yr,   1,         1,         |yr|^2_hi, |yr|^2_lo]
* Sharding: 8 cores = 4 batches x 2 halves of N. Each core owns a
  [4096, 8192] distance block.
* Per core, loop row tiles t (32 "supers") outer; per super, 8 evac chunks
  of [128, 1024] PSUM (4-deep PSUM pool decouples the PE/evac ping-pong),
  all negated+converted into one contiguous [128, 8192] fp16 super tile:
    - fused chunks (0-6 per super, ~67 of 256 total): DVE tensor_scalar
      mult(-1) + row-max accum straight from PSUM (1x mode)
    - ACT chunks (~189): activation negate+convert; then ONE wide DVE 4x
      tensor_scalar row-max ("quad") over the ACT span
  The quad + column-max ops for super t-1 are emitted AFTER super t's
  chunk-0 evac so the 4.3us TT chain step never blocks the PSUM pipeline
  in the in-order DVE queue.
  Column mins (engine-balanced, all ops verified legal on HW):
    - 15 supers: DVE tensor_tensor max into 2 running chains; the final
      chain step is split 4-ways with partial ships overlapping compute
    - 16 supers (odds<=25 + {2,14,26}): Pool partition_all_reduce into a
      dedicated 2-buf output (so ships never pin super buffers), row 0
      shipped; all ARs before t=27 so the SP queue of dependency-parked
      ships drains before the run ends
    - super 31 ships raw per-chunk (host-reduced) to kill the drain tail
  Host: max-combine 16 AR rows + 3*128 raw/chain rows per core (negated),
  7 row-max slots per super, final means in fp64.

Timeline-sim: 229.6us/core vs 239.1us baseline; all of DVE/ACT/Pool run
~85-97% busy (evacuation-bound: every PSUM element must cross to SBUF
through ACT at 0.83ns/col or DVE at 1.04ns/col - that floor is ~185us).
"""

import numpy as np
from contextlib import ExitStack

import concourse.bass_isa as bass_isa
import concourse.mybir as mybir
import concourse.tile as tile
from concourse import bacc
from concourse.bass_utils import run_bass_kernel_spmd

B, D, N, M = 4, 3, 8192, 8192
NCORES = 8
NHALF = N // 2            # rows per core
P = 128                   # partitions
NT = NHALF // P           # 32 row supers per core
MT = 512                  # matmul moving free size
CHUNK = 1024              # evac chunk (2 matmul tiles, 2 PSUM banks)
NG = M // CHUNK           # 8 evac chunks per super
KA = 7                    # augmented contraction dim (hi/lo norm splits)
NCHAIN = 2                # DVE TT column chains (over even supers)
NSLOT = 7                 # row-max slots per super (<=6 fused + 1 quad)
NAR = NT // 2             # supers reduced directly on Pool (t odd)

F32 = mybir.dt.float32
F32R = mybir.dt.float32r
F16 = mybir.dt.float16

BIG = 3.0e38

# fused-chunk count per super (1024-wide units), tuned so engine busy
# balances: ACT ~ 196 chunks, DVE fused ~ 60, TT 15 supers, Pool AR 16.
# t=0 is ACT-only: the PE p-state ramp makes the first chunks slow, and
# ACT (the busiest engine) must start as early as possible.
def _nfused(t):
    if t == 0:            # ACT covers the PE p-state ramp (chunks 0-1),
        return 6          # then 6 fused chunks start DVE early
    if t in (1, 9, 17, 25):   # ACT-only supers, spread evenly
        return 0
    if t % 4 == 3 and t != 31:   # 7 supers carry an extra fused chunk
        return 3
    return 2


# Pool-AR supers: 16 supers, all at t<=25 so Pool (and the SP queue of
# parked colres ships) drains well before the run ends; {31} ships raw
# (host-reduced) to kill the tail, the remaining supers feed DVE TT chains.
_AR_SET = frozenset(t for t in range(1, 26) if t % 2 == 1) | {2, 14, 26}
_SHIP_RAW = frozenset({31})


_cached_nc = None
last_results = None


def _build():
    """Build and compile the per-core SPMD program (same on all 8 cores)."""
    global _cached_nc
    if _cached_nc is not None:
        return _cached_nc

    nc = bacc.Bacc("TRN2", target_bir_lowering=False, debug=False,
                   num_devices=NCORES)

    xt = nc.dram_tensor("xt", [KA, NHALF], F32R, kind="ExternalInput").ap()
    yt = nc.dram_tensor("yt", [KA, M], F32R, kind="ExternalInput").ap()
    # negated row maxes: 3 slots per super t
    rowres_d = nc.dram_tensor("rowres", [P, NT * NSLOT], F32,
                              kind="ExternalOutput").ap()
    # negated col maxes from Pool-AR supers
    colres_d = nc.dram_tensor("colres", [NAR, M], F16,
                              kind="ExternalOutput").ap()
    # raw TT chains + raw-shipped supers, host-reduced over partitions
    colchain_d = nc.dram_tensor("colchain", [NCHAIN + len(_SHIP_RAW), P, M],
                                F16, kind="ExternalOutput").ap()
    _raw_row = {t: NCHAIN + i for i, t in enumerate(sorted(_SHIP_RAW))}

    mx = mybir.AluOpType.max
    mult = mybir.AluOpType.mult

    with tile.TileContext(nc) as tc, ExitStack() as ctx:
        consts = ctx.enter_context(tc.tile_pool(name="consts", bufs=1))
        accs = ctx.enter_context(tc.tile_pool(name="accs", bufs=1))
        super_pool = ctx.enter_context(tc.tile_pool(name="super", bufs=5))
        arres_pool = ctx.enter_context(tc.tile_pool(name="arres", bufs=2))
        psum_pool = ctx.enter_context(
            tc.tile_pool(name="psum", bufs=4, space="PSUM"))

        xs = consts.tile([KA, NHALF], F32R)
        nc.sync.dma_start(out=xs[:], in_=xt)
        ys = consts.tile([KA, M], F32R)
        for gd in range(8):   # split so the first matmul starts sooner
            sl = slice(gd * (M // 8), (gd + 1) * (M // 8))
            nc.sync.dma_start(out=ys[:, sl], in_=yt[:, sl])

        rmin = accs.tile([P, NT * NSLOT], F32)
        nc.gpsimd.memset(rmin[:], -BIG)   # unused slots must lose host max
        chains = [accs.tile([P, M], F16, name=f"chain{c}", tag=f"chain{c}")
                  for c in range(NCHAIN)]
        # tiny dummy ACT op: pulls the Copy act-table load into the DMA wait
        dummy = accs.tile([P, 1], F32)
        nc.gpsimd.memset(dummy[:], 0.0)
        nc.scalar.mul(dummy[:], dummy[:], 0.0)

        chain_started = [False] * NCHAIN
        chain_last = {}      # chain idx -> last super t feeding it
        tts = [t for t in range(NT)
               if t not in _AR_SET and t not in _SHIP_RAW]
        for i, t in enumerate(tts):
            chain_last[i % NCHAIN] = t
        state = {"ncolrow": 0, "nchain_t": 0, "nraw": NCHAIN}

        def finish_super(t, sup, nfused):
            """Emit the deferred row-quad + column-max ops for super t.

            Called AFTER the next super's fused evacs are emitted, so the
            4.3us TT chain step never sits in front of them in the
            in-order DVE queue (it would hold the PSUM ping-pong buffer
            and stall the PE->ACT pipeline).
            """
            if nfused < NG:
                # one wide 4x row-max over the ACT slices (in-place)
                if t == 0:
                    asl = sup[:, :(NG - nfused) * CHUNK]
                else:
                    asl = sup[:, nfused * CHUNK:]
                out = asl
                if t in _SHIP_RAW:   # don't WAR the in-flight raw ships
                    out = chains[1][:, :asl.shape[1]]
                nc.vector.tensor_scalar(
                    out, asl, -BIG, None, op0=mx, op1=mx,
                    accum_out=rmin[:, t * NSLOT + 6:t * NSLOT + 7])
            if t in _SHIP_RAW:
                pass             # shipped per-chunk during evacuation
            elif t in _AR_SET:   # Pool all-reduce into its own buffer so
                # the ship DMA never pins the super buffer
                arres = arres_pool.tile([P, M], F16, tag="arres")
                nc.gpsimd.partition_all_reduce(arres[:], sup[:], P,
                                               bass_isa.ReduceOp.max)
                i = state["ncolrow"]
                nc.sync.dma_start(out=colres_d[i:i + 1, :], in_=arres[0:1, :])
                state["ncolrow"] += 1
            else:                # DVE TT max into chain
                c = state["nchain_t"] % NCHAIN
                state["nchain_t"] += 1
                if not chain_started[c]:
                    nc.vector.tensor_copy(chains[c][:], sup[:])
                    chain_started[c] = True
                elif chain_last[c] == t:
                    # final step: split 4-ways, ship each part as it lands
                    for q in range(4):
                        qs = slice(q * (M // 4), (q + 1) * (M // 4))
                        nc.vector.tensor_tensor(chains[c][:, qs],
                                                chains[c][:, qs],
                                                sup[:, qs], op=mx)
                        nc.sync.dma_start(out=colchain_d[c][:, qs],
                                          in_=chains[c][:, qs])
                else:
                    nc.vector.tensor_tensor(chains[c][:], chains[c][:],
                                            sup[:], op=mx)

        pending = None
        for t in range(NT):
            sup = super_pool.tile([P, M], F16, tag="sup")
            nfused = _nfused(t)
            lhsT = xs[:, t * P:(t + 1) * P]          # [KA, 128] f32r
            for g in range(NG):
                ps = psum_pool.tile([P, CHUNK], F32, tag="ps")
                for j in range(CHUNK // MT):
                    m0 = g * CHUNK + j * MT
                    nc.tensor.matmul(
                        ps[:, j * MT:(j + 1) * MT], lhsT,
                        ys[:, m0:m0 + MT], start=True, stop=True)
                sl = sup[:, g * CHUNK:(g + 1) * CHUNK]
                fused = g < nfused if t != 0 else g >= NG - nfused
                si = g if t != 0 else g - (NG - nfused)
                if fused:        # DVE: negate+convert+row-max in one 1x op
                    nc.vector.tensor_scalar(
                        sl, ps[:], -1.0, None, op0=mult, op1=mx,
                        accum_out=rmin[:, t * NSLOT + si:t * NSLOT + si + 1])
                else:            # ACT: negate+convert
                    nc.scalar.mul(sl, ps[:], -1.0)
                if t in _SHIP_RAW:   # ship each chunk as soon as it lands
                    nc.sync.dma_start(
                        out=colchain_d[_raw_row[t]][:, g * CHUNK:(g + 1) * CHUNK],
                        in_=sl)
                if g == 0 and pending is not None:
                    finish_super(*pending)
                    pending = None
            pending = (t, sup, nfused)
        finish_super(*pending)
        nc.scalar.dma_start(out=rowres_d, in_=rmin[:])

    nc.compile()
    _cached_nc = nc
    return nc


def _f32r_round(a):
    """Round fp32 to the PE's f32r format: 1s + 8e + 11m (top 20 bits), RNE."""
    u = np.ascontiguousarray(a, np.float32).view(np.uint32).astype(np.uint64)
    lsb = (u >> 12) & 1
    u = ((u + 0x7FF + lsb) >> 12) << 12
    return (u & 0xFFFFFFFF).astype(np.uint32).view(np.float32)


def _augment(x, y):
    """Host-side augmentation. x,y: [B, 3, N] fp32 -> xa,ya: [B, 7, *] f32r.

    Points are pre-rounded to f32r so the PE computes the exact squared
    distance between the *rounded* points: |xr|^2 is computed from xr and
    carried as f32r hi + residual lo rows (both exactly representable up
    to ~1e-7), preserving the |xr-yr|^2 cancellation structure.
    """
    xr = _f32r_round(x)
    yr = _f32r_round(y)
    ones = np.ones((x.shape[0], 1, x.shape[2]), np.float32)

    def hilo(sq):
        hi = _f32r_round(sq)
        lo = _f32r_round(sq - hi)
        return hi[:, None, :], lo[:, None, :]

    xsq_hi, xsq_lo = hilo(np.sum(xr * xr, axis=1, dtype=np.float32))
    ysq_hi, ysq_lo = hilo(np.sum(yr * yr, axis=1, dtype=np.float32))
    xa = np.concatenate([-2.0 * xr, xsq_hi, xsq_lo, ones, ones],
                        axis=1).astype(np.float32)
    ya = np.concatenate([yr, ones, ones, ysq_hi, ysq_lo],
                        axis=1).astype(np.float32)
    return xa, ya


def kernel(x, y):
    global last_results
    x = np.ascontiguousarray(np.asarray(x, dtype=np.float32))
    y = np.ascontiguousarray(np.asarray(y, dtype=np.float32))
    assert x.shape == (B, D, N) and y.shape == (B, D, M)

    xa, ya = _augment(x, y)

    in_maps = []
    for c in range(NCORES):
        b, h = divmod(c, 2)
        in_maps.append({
            "xt": np.ascontiguousarray(xa[b, :, h * NHALF:(h + 1) * NHALF]),
            "yt": np.ascontiguousarray(ya[b]),
        })

    nc = _build()
    res = run_bass_kernel_spmd(nc, in_maps, list(range(NCORES)))
    last_results = res

    cham_x = 0.0
    cham_y = 0.0
    for b in range(B):
        r0 = res.results[2 * b]
        r1 = res.results[2 * b + 1]
        # rowres holds per-slot max(-d2); max slots -> -min(d2) per row
        row_sum = 0.0
        for r in (r0, r1):
            slots = r["rowres"].reshape(P, NT, NSLOT)
            rowmax = slots.max(axis=2)          # [P, NT] = max(-d2) per row
            row_sum -= rowmax.astype(np.float64).sum()
        # colres rows + raw chains hold partial col max(-d2); combine all
        col = np.maximum(r0["colres"].max(axis=0), r1["colres"].max(axis=0))
        for r in (r0, r1):
            col = np.maximum(col, r["colchain"].max(axis=(0, 1)))
        col_sum = -col.astype(np.float64).sum()
        cham_x += row_sum / N
        cham_y += col_sum / M
    dist = cham_x / B + cham_y / B
    return np.float32(dist)
